# revision 3
# baseline (speedup 1.0000x reference)
"""NeRF object MLP on 8 Trainium2 NeuronCores (Bass/Tile).

Strategy (hardcoded, data-parallel over points):
  - Host: compute occupancy mask, gather occupied rows, transpose inputs to
    feature-major [C, n], pad per-core to a supertile multiple, shard over 8
    cores (params replicated).
  - Device (per core): feature-major MLP. Activations are SBUF tiles
    [features<=128, n_tile]; every linear layer is a PSUM-accumulated series
    of f32r matmuls (lhsT = weight block [K,M], rhs = activation [K,N=512]).
    Bias+ReLU6 is split ACT(Relu+bias) / DVE(min 6) with a few blocks routed
    entirely to DVE to balance engine load. Outputs stay feature-major.
  - Host: scatter sigma/rgb back to full [N,1]/[N,3], empty rows get
    (-10000, 0).
"""

import math

import numpy as np

import concourse.bacc as bacc
import concourse.bass as bass
import concourse.mybir as mybir
import concourse.tile as tile
from concourse.bass_utils import run_bass_kernel_spmd

F32 = mybir.dt.float32
F32R = mybir.dt.float32r
AF = mybir.ActivationFunctionType
ALU = mybir.AluOpType

N_CORES = 8
ST = 1024  # points per supertile
MMN = 512  # matmul moving free dim (one PSUM bank of fp32)

ENC_FINS = [95, 256, 256, 256, 351, 256, 256, 256]

# (layer_key, mb) pairs whose bias+relu6 runs fully on DVE instead of
# ACT Relu + DVE min6. Balances ACT vs DVE busy time.
DVE_BLOCKS = {("enc1", 1), ("enc3", 1), ("enc5", 1), ("enc7", 1), ("dirl", 0)}

TRACE = False  # set True from test harness to capture an NTFF profile
LAST_RESULTS = None  # BassKernelResults of the most recent run (for tests)

_BUILD_CACHE: dict[int, object] = {}


def _ksplit(key, fin):
    if key == "enc4":
        return [95, 128, 128]  # concat([input_x, h]) alignment
    out = []
    r = fin
    while r > 0:
        s = min(128, r)
        out.append(s)
        r -= s
    return out


def _build(n_pts):
    nsup = n_pts // ST
    nc = bacc.Bacc("TRN2", target_bir_lowering=False)

    x0_d = nc.dram_tensor("x0", [95, n_pts], F32R, kind="ExternalInput")
    dir_d = nc.dram_tensor("dirs", [27, n_pts], F32R, kind="ExternalInput")

    layers = [(f"enc{i}", fin, 256) for i, fin in enumerate(ENC_FINS)]
    layers += [
        ("final", 256, 256),
        ("sigma", 256, 1),
        ("dirl", 283, 128),
        ("rgb", 128, 3),
    ]
    wd, bd = {}, {}
    for key, fin, fout in layers:
        wd[key] = nc.dram_tensor(f"w_{key}", [fin, fout], F32R, kind="ExternalInput")
        bd[key] = nc.dram_tensor(f"b_{key}", [fout, 1], F32, kind="ExternalInput")

    sig_d = nc.dram_tensor("sig_out", [1, n_pts], F32, kind="ExternalOutput")
    rgb_d = nc.dram_tensor("rgb_out", [3, n_pts], F32, kind="ExternalOutput")

    with tile.TileContext(nc) as tc:
        with (
            tc.tile_pool(name="wp", bufs=1) as wp,
            tc.tile_pool(name="xp", bufs=3) as xp,
            tc.tile_pool(name="hp", bufs=6) as hp,
            tc.tile_pool(name="fp", bufs=3) as fp,
            tc.tile_pool(name="op", bufs=3) as op,
            tc.tile_pool(name="pp", bufs=3, space="PSUM") as pp,
            tc.tile_pool(name="sp", bufs=2, space="PSUM") as sp,
        ):
            # ---- stage weights+biases into SBUF once ----
            W = {}  # key -> list of [ksz, fout] tiles
            B = {}  # key -> list of [mb_size, 1] tiles
            for key, fin, fout in layers:
                tiles = []
                k0 = 0
                for j, ksz in enumerate(_ksplit(key, fin)):
                    wt = wp.tile([ksz, fout], F32R, name=f"w_{key}_{j}",
                                 tag=f"w_{key}_{j}")
                    nc.sync.dma_start(wt[:, :], wd[key][k0:k0 + ksz, :])
                    tiles.append(wt)
                    k0 += ksz
                W[key] = tiles
                bts = []
                for mb in range(math.ceil(fout / 128)):
                    msz = min(128, fout - mb * 128)
                    bt = wp.tile([msz, 1], F32, name=f"b_{key}_{mb}",
                                 tag=f"b_{key}_{mb}")
                    nc.sync.dma_start(bt[:, :], bd[key][mb * 128:mb * 128 + msz, :])
                    bts.append(bt)
                B[key] = bts

            def linear(key, srcs, fout, act, pool, tag, s, odt=F32R):
                """srcs: list of (ksz, ap[ksz, ST]). Returns list of out tiles."""
                outs = []
                for mb in range(math.ceil(fout / 128)):
                    msz = min(128, fout - mb * 128)
                    if msz == 128:
                        ps = pp.tile([128, ST], F32, name=f"ps_{key}{mb}_{s}",
                                     tag="ps")
                        psub = [ps[:, u * MMN:(u + 1) * MMN]
                                for u in range(ST // MMN)]
                    else:
                        ps = None
                        psub = [sp.tile([msz, MMN], F32,
                                        name=f"pss_{key}{mb}_{s}_{u}", tag="pss")
                                for u in range(ST // MMN)]
                    for u in range(ST // MMN):
                        nk = len(srcs)
                        for j, (ksz, src) in enumerate(srcs):
                            nc.tensor.matmul(
                                psub[u][:msz, :],
                                W[key][j][:, mb * 128:mb * 128 + msz],
                                src[:, u * MMN:(u + 1) * MMN],
                                start=(j == 0),
                                stop=(j == nk - 1),
                            )
                    ot = pool.tile([msz, ST], odt, name=f"{tag}{mb}_{s}",
                                   tag=f"{tag}{mb}")
                    bias = B[key][mb][:, :]
                    for u, pu in enumerate(
                        [ps[:, :]] if ps is not None else psub
                    ):
                        osl = ot[:, :] if ps is not None else \
                            ot[:, u * MMN:(u + 1) * MMN]
                        if act == "relu6":
                            if (key, mb) in DVE_BLOCKS:
                                nc.vector.tensor_scalar(
                                    osl, pu, bias, 0.0, ALU.add, ALU.max)
                            else:
                                nc.scalar.activation(osl, pu, AF.Relu, bias=bias)
                            nc.vector.tensor_scalar(osl, osl, 6.0, None, ALU.min)
                        elif act == "ident":
                            nc.scalar.activation(osl, pu, AF.Identity, bias=bias)
                        elif act == "sigmoid":
                            nc.scalar.activation(osl, pu, AF.Sigmoid, bias=bias)
                    outs.append(ot)
                return outs

            for s in range(nsup):
                sl = bass.ts(s, ST)
                x0 = xp.tile([95, ST], F32R, name=f"x0_{s}", tag="x0")
                nc.sync.dma_start(x0[:, :], x0_d[:, sl])
                dr = xp.tile([27, ST], F32R, name=f"dr_{s}", tag="dr")
                nc.sync.dma_start(dr[:, :], dir_d[:, sl])

                h = linear("enc0", [(95, x0)], 256, "relu6", hp, "h", s)
                for i in range(1, 8):
                    srcs = [(128, h[0]), (128, h[1])]
                    if i == 4:
                        srcs = [(95, x0)] + srcs
                    h = linear(f"enc{i}", srcs, 256, "relu6", hp, "h", s)

                hsrc = [(128, h[0]), (128, h[1])]
                sig = linear("sigma", hsrc, 1, "ident", op, "sg", s, odt=F32)
                xf = linear("final", hsrc, 256, "ident", fp, "xf", s)
                d = linear("dirl", [(128, xf[0]), (128, xf[1]), (27, dr)],
                           128, "relu6", fp, "d", s)
                rgb = linear("rgb", [(128, d[0])], 3, "sigmoid", op, "rg", s, odt=F32)

                nc.sync.dma_start(sig_d[:, sl], sig[0][:, :])
                nc.sync.dma_start(rgb_d[:, sl], rgb[0][:, :])

    nc.compile()
    return nc


def _get_nc(n_pts):
    if n_pts not in _BUILD_CACHE:
        _BUILD_CACHE[n_pts] = _build(n_pts)
    return _BUILD_CACHE[n_pts]


def kernel(xyz_embedded, inst_embedded, input_dir, params):
    global LAST_RESULTS
    xyz = np.ascontiguousarray(np.asarray(xyz_embedded, dtype=np.float32))
    inst = np.ascontiguousarray(np.asarray(inst_embedded, dtype=np.float32))
    dirs = np.ascontiguousarray(np.asarray(input_dir, dtype=np.float32))
    n_full = xyz.shape[0]

    occ = np.any(xyz != 0.0, axis=1)
    idx = np.flatnonzero(occ)
    m = idx.size
    npc = max(1, math.ceil(m / N_CORES))  # rows per core (unpadded)
    npp = max(ST, math.ceil(npc / ST) * ST)  # padded rows per core

    weights = {}
    for key, (w, b) in zip(
        [f"enc{i}" for i in range(8)] + ["final", "sigma", "dirl", "rgb"],
        list(params["enc"]) + [params["final"], params["sigma"],
                               params["dir"], params["rgb"]],
    ):
        weights[f"w_{key}"] = np.ascontiguousarray(np.asarray(w, np.float32))
        weights[f"b_{key}"] = np.ascontiguousarray(
            np.asarray(b, np.float32).reshape(-1, 1))

    in_maps = []
    core_rows = []
    for c in range(N_CORES):
        rows = idx[c * npc:min((c + 1) * npc, m)]
        core_rows.append(rows)
        x0 = np.zeros((95, npp), np.float32)
        dd = np.zeros((27, npp), np.float32)
        k = rows.size
        if k:
            x0[:63, :k] = xyz[rows].T
            x0[63:, :k] = inst[rows].T
            dd[:, :k] = dirs[rows].T
        im = {"x0": x0, "dirs": dd}
        im.update(weights)
        in_maps.append(im)

    nc = _get_nc(npp)
    res = run_bass_kernel_spmd(
        nc, in_maps, core_ids=list(range(N_CORES)), trace=TRACE)
    LAST_RESULTS = res

    full_sigma = np.full((n_full, 1), -10000.0, np.float32)
    full_rgb = np.zeros((n_full, 3), np.float32)
    for c, rows in enumerate(core_rows):
        k = rows.size
        if k:
            full_sigma[rows, 0] = res.results[c]["sig_out"][0, :k]
            full_rgb[rows] = res.results[c]["rgb_out"][:, :k].T
    return full_sigma, full_rgb


# revision 11
# speedup vs baseline: 30.7563x; 30.7563x over previous
"""NeRF object MLP on 8 Trainium2 NeuronCores (Bass/Tile).

Strategy (hardcoded, data-parallel over points):
  - Host: compute occupancy mask, gather occupied rows, transpose inputs to
    feature-major [C, n], pad per-core to a supertile multiple, shard over 8
    cores (params replicated).
  - Device (per core): feature-major MLP. Activations are SBUF tiles
    [features<=128, n_tile]; every linear layer is a PSUM-accumulated series
    of f32r matmuls (lhsT = weight block [K,M], rhs = activation [K,N=512]).
    Bias+ReLU6 is split ACT(Relu+bias) / DVE(min 6) with a few blocks routed
    entirely to DVE to balance engine load. Outputs stay feature-major.
  - Host: scatter sigma/rgb back to full [N,1]/[N,3], empty rows get
    (-10000, 0).
"""

import math

import numpy as np

import concourse.bacc as bacc
import concourse.bass as bass
import concourse.mybir as mybir
import concourse.tile as tile
from concourse.bass_utils import run_bass_kernel_spmd

F32 = mybir.dt.float32
F32R = mybir.dt.float32r
AF = mybir.ActivationFunctionType
ALU = mybir.AluOpType

N_CORES = 8
ST = 1024  # points per supertile
MMN = 512  # matmul moving free dim (one PSUM bank of fp32)

# fin includes a ones-row for enc0/enc4/dirl: bias rides in the matmul
ENC_FINS = [96, 256, 256, 256, 352, 256, 256, 256]
ONES_KEYS = {"enc0", "enc4", "dirl"}

# (layer_key, mb) pairs whose bias+relu6 runs fully on DVE instead of
# ACT Relu + DVE min6. Balances ACT vs DVE busy time.
DVE_BLOCKS = set()

TRACE = False  # set True from test harness to capture an NTFF profile
LAST_RESULTS = None  # BassKernelResults of the most recent run (for tests)

_BUILD_CACHE: dict[int, object] = {}


def _ksplit(key, fin):
    if key == "enc4":
        return [96, 128, 128]  # concat([input_x+ones, h]) alignment
    if key == "dirl":
        return [128, 128, 28]  # concat([x_final, dirs+ones])
    out = []
    r = fin
    while r > 0:
        s = min(128, r)
        out.append(s)
        r -= s
    return out


def _build(n_pts):
    nsup = n_pts // ST
    nc = bacc.Bacc("TRN2", target_bir_lowering=False)

    x0_d = nc.dram_tensor("x0", [96, n_pts], F32R, kind="ExternalInput")
    dir_d = nc.dram_tensor("dirs", [28, n_pts], F32R, kind="ExternalInput")

    layers = [(f"enc{i}", fin, 256) for i, fin in enumerate(ENC_FINS)]
    layers += [
        ("final", 256, 256),
        ("sigma", 256, 1),
        ("dirl", 284, 128),
        ("rgb", 128, 3),
    ]
    wd, bd = {}, {}
    for key, fin, fout in layers:
        wd[key] = nc.dram_tensor(f"w_{key}", [fin, fout], F32R, kind="ExternalInput")
        if key not in ONES_KEYS:
            bd[key] = nc.dram_tensor(f"b_{key}", [fout, 1], F32, kind="ExternalInput")

    sig_d = nc.dram_tensor("sig_out", [1, n_pts], F32, kind="ExternalOutput")
    rgb_d = nc.dram_tensor("rgb_out", [3, n_pts], F32, kind="ExternalOutput")

    with tile.TileContext(nc) as tc:
        with (
            tc.tile_pool(name="wp", bufs=1) as wp,
            tc.tile_pool(name="xp", bufs=4) as xp,
            tc.tile_pool(name="hp", bufs=8) as hp,
            tc.tile_pool(name="fp", bufs=3) as fp,
            tc.tile_pool(name="op", bufs=3) as op,
            tc.tile_pool(name="pp", bufs=4, space="PSUM") as pp,
        ):
            # ---- stage weights+biases into SBUF once ----
            W = {}  # key -> list of [ksz, fout] tiles
            B = {}  # key -> list of [mb_size, 1] tiles
            for key, fin, fout in layers:
                tiles = []
                k0 = 0
                for j, ksz in enumerate(_ksplit(key, fin)):
                    wt = wp.tile([ksz, fout], F32R, name=f"w_{key}_{j}",
                                 tag=f"w_{key}_{j}")
                    nc.gpsimd.dma_start(wt[:, :], wd[key][k0:k0 + ksz, :])
                    tiles.append(wt)
                    k0 += ksz
                W[key] = tiles
                bts = []
                if key in ONES_KEYS:
                    B[key] = []
                    continue
                for mb in range(math.ceil(fout / 128)):
                    msz = min(128, fout - mb * 128)
                    bt = wp.tile([msz, 1], F32, name=f"b_{key}_{mb}",
                                 tag=f"b_{key}_{mb}")
                    nc.gpsimd.dma_start(bt[:, :], bd[key][mb * 128:mb * 128 + msz, :])
                    bts.append(bt)
                B[key] = bts

            def linear(key, srcs, fout, act, pool, tag, s, odt=F32R):
                """srcs: list of (ksz, ap[ksz, ST]). Returns list of out tiles."""
                outs = []
                for mb in range(math.ceil(fout / 128)):
                    msz = min(128, fout - mb * 128)
                    ps = pp.tile([msz, ST], F32, name=f"ps_{key}{mb}_{s}",
                                 tag="ps", padded_shape=[128, ST])
                    psub = [ps[:, u * MMN:(u + 1) * MMN]
                            for u in range(ST // MMN)]
                    for u in range(ST // MMN):
                        nk = len(srcs)
                        for j, (ksz, src) in enumerate(srcs):
                            nc.tensor.matmul(
                                psub[u][:msz, :],
                                W[key][j][:, mb * 128:mb * 128 + msz],
                                src[:, u * MMN:(u + 1) * MMN],
                                start=(j == 0),
                                stop=(j == nk - 1),
                            )
                    ot = pool.tile([msz, ST], odt, name=f"{tag}{mb}_{s}",
                                   tag=f"{tag}{mb}")
                    bias = B[key][mb][:, :] if B[key] else None
                    for u, pu in enumerate([ps[:, :]]):
                        osl = ot[:, :]
                        if act == "relu6":
                            if key in ONES_KEYS:
                                nc.vector.tensor_scalar(
                                    osl, pu, 0.0, 6.0, ALU.max, ALU.min)
                                continue
                            if (key, mb) in DVE_BLOCKS:
                                nc.vector.tensor_scalar(
                                    osl, pu, bias, 0.0, ALU.add, ALU.max)
                            else:
                                nc.scalar.activation(osl, pu, AF.Relu, bias=bias)
                            nc.vector.tensor_scalar(osl, osl, 6.0, None, ALU.min)
                        elif act == "ident":
                            nc.scalar.activation(osl, pu, AF.Identity, bias=bias)
                        elif act == "sigmoid":
                            nc.scalar.activation(osl, pu, AF.Sigmoid, bias=bias)
                    outs.append(ot)
                return outs

            # two independent point streams, interleaved layer-by-layer so
            # each stream's matmuls fill the other's ACT/DVE tail gaps.
            # pairs are software-pipelined: pair t's head (enc chain) is
            # emitted before pair t-1's tail, so PSUM slot recycling never
            # makes a new pair wait on the previous pair's tail drains.
            def emit_head(t):
                pair = (2 * t, 2 * t + 1)
                x0s, drs, hs = {}, {}, {}
                for s in pair:
                    sl = bass.ts(s, ST)
                    x0 = xp.tile([96, ST], F32R, name=f"x0_{s}", tag="x0")
                    nc.sync.dma_start(x0[:, :], x0_d[:, sl])
                    dr = xp.tile([28, ST], F32R, name=f"dr_{s}", tag="dr")
                    nc.sync.dma_start(dr[:, :], dir_d[:, sl])
                    x0s[s], drs[s] = x0, dr
                for s in pair:
                    hs[s] = linear("enc0", [(96, x0s[s])], 256, "relu6",
                                   hp, "h", s)
                for i in range(1, 8):
                    for s in pair:
                        h = hs[s]
                        srcs = [(128, h[0]), (128, h[1])]
                        if i == 4:
                            srcs = [(96, x0s[s])] + srcs
                        hs[s] = linear(f"enc{i}", srcs, 256, "relu6",
                                       hp, "h", s)
                return pair, x0s, drs, hs

            def emit_tail(state):
                pair, x0s, drs, hs = state
                sigs, xfs, ds = {}, {}, {}
                for s in pair:
                    hsrc = [(128, hs[s][0]), (128, hs[s][1])]
                    sigs[s] = linear("sigma", hsrc, 1, "ident", op, "sg", s,
                                     odt=F32)
                    xfs[s] = linear("final", hsrc, 256, "ident", fp, "xf", s)
                for s in pair:
                    xf = xfs[s]
                    ds[s] = linear("dirl",
                                   [(128, xf[0]), (128, xf[1]), (28, drs[s])],
                                   128, "relu6", fp, "d", s)
                for s in pair:
                    rgb = linear("rgb", [(128, ds[s][0])], 3, "sigmoid",
                                 op, "rg", s, odt=F32)
                    sl = bass.ts(s, ST)
                    nc.gpsimd.dma_start(sig_d[:, sl], sigs[s][0][:, :])
                    nc.gpsimd.dma_start(rgb_d[:, sl], rgb[0][:, :])

            pending = None
            for t in range(nsup // 2):
                state = emit_head(t)
                if pending is not None:
                    emit_tail(pending)
                pending = state
            if pending is not None:
                emit_tail(pending)

    nc.compile()
    return nc


def _get_nc(n_pts):
    if n_pts not in _BUILD_CACHE:
        _BUILD_CACHE[n_pts] = _build(n_pts)
    return _BUILD_CACHE[n_pts]


def kernel(xyz_embedded, inst_embedded, input_dir, params):
    global LAST_RESULTS
    xyz = np.ascontiguousarray(np.asarray(xyz_embedded, dtype=np.float32))
    inst = np.ascontiguousarray(np.asarray(inst_embedded, dtype=np.float32))
    dirs = np.ascontiguousarray(np.asarray(input_dir, dtype=np.float32))
    n_full = xyz.shape[0]

    occ = np.any(xyz != 0.0, axis=1)
    idx = np.flatnonzero(occ)
    m = idx.size
    npc = max(1, math.ceil(m / N_CORES))  # rows per core (unpadded)
    npp = max(2 * ST, math.ceil(npc / (2 * ST)) * 2 * ST)  # padded rows per core

    weights = {}
    for key, (w, b) in zip(
        [f"enc{i}" for i in range(8)] + ["final", "sigma", "dirl", "rgb"],
        list(params["enc"]) + [params["final"], params["sigma"],
                               params["dir"], params["rgb"]],
    ):
        w = np.asarray(w, np.float32)
        b = np.asarray(b, np.float32)
        if key == "enc0":
            w = np.vstack([w, b.reshape(1, -1)])
        elif key == "enc4":
            w = np.vstack([w[0:95], b.reshape(1, -1), w[95:351]])
        elif key == "dirl":
            w = np.vstack([w, b.reshape(1, -1)])
        else:
            weights[f"b_{key}"] = np.ascontiguousarray(b.reshape(-1, 1))
        weights[f"w_{key}"] = np.ascontiguousarray(w)

    in_maps = []
    core_rows = []
    for c in range(N_CORES):
        rows = idx[c * npc:min((c + 1) * npc, m)]
        core_rows.append(rows)
        x0 = np.zeros((96, npp), np.float32)
        dd = np.zeros((28, npp), np.float32)
        x0[95, :] = 1.0
        dd[27, :] = 1.0
        k = rows.size
        if k:
            x0[:63, :k] = xyz[rows].T
            x0[63:95, :k] = inst[rows].T
            dd[:27, :k] = dirs[rows].T
        im = {"x0": x0, "dirs": dd}
        im.update(weights)
        in_maps.append(im)

    nc = _get_nc(npp)
    res = run_bass_kernel_spmd(
        nc, in_maps, core_ids=list(range(N_CORES)), trace=TRACE)
    LAST_RESULTS = res

    full_sigma = np.full((n_full, 1), -10000.0, np.float32)
    full_rgb = np.zeros((n_full, 3), np.float32)
    for c, rows in enumerate(core_rows):
        k = rows.size
        if k:
            full_sigma[rows, 0] = res.results[c]["sig_out"][0, :k]
            full_rgb[rows] = res.results[c]["rgb_out"][:, :k].T
    return full_sigma, full_rgb
